# revision 1
# baseline (speedup 1.0000x reference)
"""NeuralMMU Trainium2 kernel.

Pipeline per core (131072 addrs, 64 iterations x 2048 addrs):
  1. SP-triggered DMA of host-unpacked bit planes -> SBUF [96, 8192] u8
     (4 iters per DMA); partition q = 32s + k holds bit k (replicated 3x,
     s = 0..2), col j*2048 + 512g + c -> addr of iter j, block g
  2. DVE tensor_copy u8 -> bf16 bits [96, 2048] per iter
  3. 4x bf16 matmul k=96: bits @ (W1hi; W1mid; W1lo) -> PSUM [128,2048]
     (exact 3-way bf16 split of f32 W1, summed in the contraction dim)
  4. ACT Gelu(+b1): PSUM -> SBUF h [128,2048]
  5. 4x f32 matmul (PE col tiles 32g): h @ W2ext -> PSUM [128,512]
  6. DVE is_gt per-partition threshold (0.5 - b2): -> bf16 bits
  7. ONE bf16 matmul, block-diagonal [128,8] weights: packs all 4
     col-bands' 26 bits as lo13/hi13 in a single 512-row pass -> PSUM
  8. DVE copy PSUM -> SBUF accumulator [8,4096] (8 iters)
  9. 1x SP-triggered DMA [8,4096] per 8 iters -> DRAM;
     host combines lo + 8192*hi -> int64

The loop is software-pipelined two-deep so the PE never stalls:
PE order per iter t is L1(t+1), L2(t), pack(t-1); DVE converts bits
for t+2 while ACT runs Gelu(t) and PE runs L2(t).  This hides both
the L1(t)->Gelu(t)->L2(t) chain (Gelu finishes ~2.4us before L2
needs it) and the L2(t)->threshold(t)->pack(t) chain (threshold has
a full iteration of slack).  A small iter-0-only input DMA (R0t)
hides most of the first group-DMA latency at startup.

PE busy is ~96% of total; the f32 L2 (4 cyc/row) is optimal for the
required exactness: logit threshold gaps go down to 2.5e-8, so the
contraction must be f32-exact, and an explicit 5-pair bf16 split
would move 10240 rows/iter vs f32's effective 8192.

HW-validated: ~299 us/core, 1/1048576 mismatch (the one addr with a
2.5e-8 logit-threshold gap; same flip as a pure-f32 kernel).
"""

import numpy as np
from contextlib import ExitStack

import concourse.bass as bass
import concourse.mybir as mybir
import concourse.tile as tile
from concourse import bacc, bass_utils

B = 1_048_576
NCORES = 8
PER = B // NCORES          # 131072 addrs per core
BLK = 512                  # addrs per PE block
NBLK = 4                   # blocks per iteration
CHUNK = NBLK * BLK         # 2048 addrs per iteration
N_ITERS = PER // CHUNK     # 64
GIN = 4                    # iters per input DMA
GOUT = 8                   # iters per output DMA set

F32 = mybir.dt.float32
BF16 = mybir.dt.bfloat16
U8 = mybir.dt.uint8
AF = mybir.ActivationFunctionType
ALU = mybir.AluOpType


def build_nc(n_iters: int = N_ITERS, act=AF.Gelu) -> bass.Bass:
    nc = bacc.Bacc("TRN2")
    assert n_iters % GOUT == 0 and n_iters % GIN == 0

    bp = nc.dram_tensor("bp", [n_iters // GIN, 96, GIN * CHUNK], U8,
                        kind="ExternalInput")
    cst_d = nc.dram_tensor("cst", [128, 102], F32, kind="ExternalInput")
    outp = nc.dram_tensor("outp", [2 * NBLK, n_iters // GOUT, GOUT * BLK], F32,
                          kind="ExternalOutput")

    with ExitStack() as ctx:
        tc = ctx.enter_context(tile.TileContext(nc))
        const = ctx.enter_context(tc.tile_pool(name="const", bufs=1))
        rpool = ctx.enter_context(tc.tile_pool(name="rp", bufs=2))
        bitsp = ctx.enter_context(tc.tile_pool(name="bitsp", bufs=2))
        hp = ctx.enter_context(tc.tile_pool(name="hp", bufs=2))
        bop = ctx.enter_context(tc.tile_pool(name="bop", bufs=2))
        pksp = ctx.enter_context(tc.tile_pool(name="pksp", bufs=2))
        hprep = ctx.enter_context(tc.tile_pool(name="hprep", bufs=1, space="PSUM"))
        l2p = ctx.enter_context(tc.tile_pool(name="l2p", bufs=2, space="PSUM"))
        pkp = ctx.enter_context(tc.tile_pool(name="pkp", bufs=2, space="PSUM"))

        cst = const.tile([128, 102], F32)
        nc.sync.dma_start(cst[:], cst_d[:])
        w1b = cst[:, 0:64].bitcast(BF16)     # [128, 128] bf16; rows 0-95 used
        w2s = cst[:, 64:96]
        b1c = cst[:, 96:97]
        thc = cst[:, 97:98]
        pwc = cst[:, 98:102].bitcast(BF16)   # [128, 8] block-diag pack weights

        R = None
        pks = None

        def load_input(t):
            nonlocal R
            if t % GIN == 0:
                R = rpool.tile([96, GIN * CHUNK], U8)
                nc.sync.dma_start(R[:], bp[t // GIN])

        def convert(t):
            bits = bitsp.tile([96, CHUNK], BF16)
            nc.vector.tensor_copy(
                bits[:], R[:, CHUNK * (t % GIN):CHUNK * (t % GIN + 1)]
            )
            return bits

        def l1mm(bits):
            hpre = hprep.tile([128, CHUNK], F32)
            for g in range(NBLK):
                nc.tensor.matmul(
                    hpre[:, BLK * g:BLK * (g + 1)],
                    w1b[0:96, :],
                    bits[0:96, BLK * g:BLK * (g + 1)],
                    start=True, stop=True, tile_position=(0, 0),
                )
            return hpre

        R0t = rpool.tile([96, CHUNK], U8)
        nc.sync.dma_start(R0t[:], bp[0, :, 0:CHUNK])
        load_input(0)
        bits0 = bitsp.tile([96, CHUNK], BF16)
        nc.vector.tensor_copy(bits0[:], R0t[:])
        hpre = l1mm(bits0)
        if n_iters > 1:
            bits_next = convert(1)

        bo_prev = None

        def pack_and_store(tp):
            nonlocal pks
            pk = pkp.tile([2 * NBLK, BLK], F32)
            nc.tensor.matmul(
                pk[:],
                pwc[:],
                bo_prev[:],
                start=True, stop=True, tile_position=(0, 0),
            )
            if tp % GOUT == 0:
                pks = pksp.tile([2 * NBLK, GOUT * BLK], F32)
            nc.vector.tensor_copy(
                pks[:, BLK * (tp % GOUT):BLK * (tp % GOUT + 1)], pk[:]
            )
            if tp % GOUT == GOUT - 1:
                nc.sync.dma_start(outp[:, tp // GOUT, :], pks[:])

        for t in range(n_iters):
            h = hp.tile([128, CHUNK], F32)
            nc.scalar.activation(h[:], hpre[:], act, bias=b1c, scale=1.0)

            if t + 2 < n_iters:
                load_input(t + 2)
                bits_fut = convert(t + 2)

            if t + 1 < n_iters:
                hpre = l1mm(bits_next)
                if t + 2 < n_iters:
                    bits_next = bits_fut

            l2o = l2p.tile([128, BLK], F32)
            for g in range(NBLK):
                nc.tensor.matmul(
                    l2o[32 * g:32 * (g + 1), :],
                    w2s[:],
                    h[:, BLK * g:BLK * (g + 1)],
                    start=True, stop=True, tile_position=(0, 32 * g),
                )

            if t > 0:
                pack_and_store(t - 1)

            bo = bop.tile([128, BLK], BF16)
            nc.vector.tensor_scalar(
                bo[:], l2o[:], thc, None, op0=ALU.is_gt,
            )
            bo_prev = bo

        pack_and_store(n_iters - 1)

    return nc


def make_const_inputs(W1, b1, W2, b2):
    import ml_dtypes

    w1 = np.ascontiguousarray(W1[0:32, :], dtype=np.float32)
    hi = w1.astype(ml_dtypes.bfloat16)
    mid = (w1 - hi.astype(np.float32)).astype(ml_dtypes.bfloat16)
    lo = (w1 - hi.astype(np.float32) - mid.astype(np.float32)).astype(
        ml_dtypes.bfloat16
    )
    w1b = np.zeros((128, 128), dtype=ml_dtypes.bfloat16)
    w1b[0:32] = hi
    w1b[32:64] = mid
    w1b[64:96] = lo

    w2s = np.zeros((128, 32), dtype=np.float32)
    w2s[:, :26] = W2[:, :26]
    b1c = np.asarray(b1, dtype=np.float32).reshape(128, 1)
    thc = np.full((128, 1), 1e30, dtype=np.float32)
    pwc = np.zeros((128, 8), dtype=np.float32)
    for g in range(4):
        thc[32 * g:32 * g + 26, 0] = 0.5 - np.asarray(b2[:26], dtype=np.float32)
        for i in range(13):
            pwc[32 * g + i, 2 * g] = float(1 << i)
            pwc[32 * g + 13 + i, 2 * g + 1] = float(1 << i)
    cst = np.empty((128, 102), dtype=np.float32)
    cst[:, 0:64] = np.ascontiguousarray(w1b).view(np.float32)
    cst[:, 64:96] = w2s
    cst[:, 96:97] = b1c
    cst[:, 97:98] = thc
    cst[:, 98:102] = (
        np.ascontiguousarray(pwc.astype(ml_dtypes.bfloat16)).view(np.float32)
    )
    return {"cst": cst}


def make_bit_planes(virtual_addr, n_iters: int = N_ITERS):
    """Per-core [n_iters//GIN, 96, GIN*2048] u8 0/1 bit-plane arrays.

    Partition 32s + k (s = 0..2 replication), col j*2048 + 512g + c =
    bit k of addr (GIN*tt + j)*2048 + g*512 + c.
    """
    va32 = np.asarray(virtual_addr).astype(np.uint32)
    per = n_iters * CHUNK
    ncores = va32.size // per
    out = []
    for c in range(ncores):
        seg = va32[c * per:(c + 1) * per]
        byt = seg.view(np.uint8).reshape(n_iters // GIN, GIN, NBLK, BLK, 4)
        bits = np.unpackbits(byt, axis=-1, bitorder="little")
        # (tt, j, g, c, k) -> (tt, k, j, g, c)
        pl = bits.transpose(0, 4, 1, 2, 3).reshape(n_iters // GIN, 32, GIN * CHUNK)
        out.append(np.ascontiguousarray(np.concatenate([pl, pl, pl], axis=1)))
    return out


def combine_output(o, n_iters: int = N_ITERS):
    """[8, n_iters//GOUT, GOUT*512] f32 -> [per] int64."""
    arr = o.reshape(NBLK, 2, n_iters // GOUT, GOUT, BLK)
    lo = arr[:, 0].transpose(1, 2, 0, 3).reshape(-1).astype(np.int64)
    hi = arr[:, 1].transpose(1, 2, 0, 3).reshape(-1).astype(np.int64)
    return lo + 8192 * hi


_NC_CACHE = {}
TRACE = False
LAST_RES = None


def kernel(virtual_addr, W1, b1, W2, b2):
    global LAST_RES
    if "nc" not in _NC_CACHE:
        nc = build_nc(N_ITERS)
        nc.finalize()
        _NC_CACHE["nc"] = nc
    nc = _NC_CACHE["nc"]

    consts = make_const_inputs(W1, b1, W2, b2)
    planes = make_bit_planes(virtual_addr, N_ITERS)
    in_maps = [{"bp": planes[c], **consts} for c in range(NCORES)]

    res = bass_utils.run_bass_kernel_spmd(
        nc, in_maps, list(range(NCORES)), trace=TRACE
    )
    LAST_RES = res

    outs = [combine_output(res.results[c]["outp"]) for c in range(NCORES)]
    return np.concatenate(outs)



# revision 6
# speedup vs baseline: 1.5769x; 1.5769x over previous
"""NeuralMMU Trainium2 kernel (v2 — ACT-bound pipeline).

Per core: 131072 addrs, 128 chunks x 1024 addrs.

  1. Host sends bit planes as bf16 (0/1), replicated x3 for the exact
     3-way bf16 split of W1 -> SBUF [96, 8192] per 8-chunk DMA group.
  2. L1: 2 bf16 matmuls k=96 per chunk: bits @ (W1hi;W1mid;W1lo)
     -> PSUM hpre [128, 1024] f32 (exact).
  3. ACT Gelu(+b1): PSUM -> SBUF h [128, 1024] f32.  This is the
     bottleneck engine: (1024 + 222) cyc @ 1.2 GHz per chunk.
  4. L2 TRANSPOSED: 8 tiles per chunk; h[:, 128g:128g+128] is the f32
     STATIONARY, W2[:, :26] f32 is the tiny MOVING operand (N=26, so
     the fp32 4 cyc/row penalty applies to only 26 columns) ->
     l2o [128 addr, 8*26] f32 PSUM, fully exact f32.
  5. DVE is_gt vs theta=(0.5-b2) broadcast -> bits bf16 [128, 208]
  6. GPSIMD multiply by 2^j pattern (13-bit halves) -> tmp [128, 208]
  7. DVE segmented tensor_reduce [128,16,13] -> [128,16] f32 packed
     lo13/hi13 per addr; 8 chunks accumulate -> [128, 128] -> DMA out.
     Host combines lo + 8192*hi.

PSUM: hpre 2 bufs x 2 banks + l2o 2 bufs x 1 bank = 6 of 8 banks.
Pipeline: L1(t+1) runs on PE during ACT(t); L2(t) waits on ACT(t).
hpre double-buffering keeps ACT back-to-back: period ~= ACT busy.
"""

import numpy as np
from contextlib import ExitStack

import concourse.bass as bass
import concourse.mybir as mybir
import concourse.tile as tile
from concourse import bacc, bass_utils

B = 1_048_576
NCORES = 8
PER = B // NCORES          # 131072 addrs per core
CHUNK = 1024               # addrs per chunk
N_ITERS = PER // CHUNK     # 128
GIN = 8                    # chunks per input DMA group
GOUT = 8                   # chunks per output DMA
NT = CHUNK // 128          # 8 L2 tiles per chunk
NBITS = 26
SEG = 13                   # packed in lo13/hi13 halves

F32 = mybir.dt.float32
BF16 = mybir.dt.bfloat16
AF = mybir.ActivationFunctionType
ALU = mybir.AluOpType
AX = mybir.AxisListType

# cst columns (f32): w1b bf16 [128,128] = 64 | w2e f32 26 | theta f32 26
# | pw2 bf16 [128,13] -> 7 (13 bf16 + pad) | b1 1
C_W1 = 0
C_W2 = 64
C_TH = 90
C_PW = 116
C_B1 = 123
NCST = 124


def build_nc(n_iters: int = N_ITERS) -> bass.Bass:
    nc = bacc.Bacc("TRN2")
    assert n_iters % GIN == 0 and n_iters % GOUT == 0

    bp = nc.dram_tensor("bp", [n_iters // GIN, 96, GIN * CHUNK], BF16,
                        kind="ExternalInput")
    cst_d = nc.dram_tensor("cst", [128, NCST], F32, kind="ExternalInput")
    outp = nc.dram_tensor("outp", [n_iters // GOUT, 128, GOUT * 2 * NT], F32,
                          kind="ExternalOutput")

    with ExitStack() as ctx:
        tc = ctx.enter_context(tile.TileContext(nc))
        const = ctx.enter_context(tc.tile_pool(name="const", bufs=1))
        rpool = ctx.enter_context(tc.tile_pool(name="rp", bufs=2))
        hp = ctx.enter_context(tc.tile_pool(name="hp", bufs=2))
        bitp = ctx.enter_context(tc.tile_pool(name="bitp", bufs=2))
        tmpp = ctx.enter_context(tc.tile_pool(name="tmpp", bufs=2))
        oaccp = ctx.enter_context(tc.tile_pool(name="oaccp", bufs=2))
        hprep = ctx.enter_context(tc.tile_pool(name="hprep", bufs=2, space="PSUM"))
        l2p = ctx.enter_context(tc.tile_pool(name="l2p", bufs=2, space="PSUM"))

        cst = const.tile([128, NCST], F32)
        nc.sync.dma_start(cst[:], cst_d[:])
        w1b = cst[:, C_W1:C_W1 + 64].bitcast(BF16)   # [128,128]; rows 0-95
        w2e = cst[:, C_W2:C_W2 + NBITS]              # [128, 26] f32
        theta = cst[:, C_TH:C_TH + NBITS]            # [128, 26] f32
        pw2 = cst[:, C_PW:C_PW + 7].bitcast(BF16)[:, 0:SEG]   # [128, 13]
        b1c = cst[:, C_B1:C_B1 + 1]

        theta_b = theta.unsqueeze(1).broadcast_to([128, NT, NBITS])
        pw2_b = pw2.unsqueeze(1).broadcast_to([128, 2 * NT, SEG])

        Rtiles = {}

        def load_input(g):
            R = rpool.tile([96, GIN * CHUNK], BF16)
            Rtiles[g] = R
            if g - 2 in Rtiles:
                del Rtiles[g - 2]
            if g == 0:
                # split first group into per-chunk DMAs so chunk 0 can
                # start without waiting for the full 1.5MB transfer
                for j in range(GIN):
                    nc.sync.dma_start(
                        R[:, CHUNK * j:CHUNK * (j + 1)],
                        bp[0, :, CHUNK * j:CHUNK * (j + 1)],
                    )
            else:
                nc.sync.dma_start(R[:], bp[g])

        def l1mm(t):
            hpre = hprep.tile([128, CHUNK], F32)
            R = Rtiles[t // GIN]
            off = CHUNK * (t % GIN)
            for b in range(CHUNK // 512):
                nc.tensor.matmul(
                    hpre[:, 512 * b:512 * (b + 1)],
                    w1b[0:96, :],
                    R[0:96, off + 512 * b:off + 512 * (b + 1)],
                    start=True, stop=True, tile_position=(0, 0),
                )
            return hpre

        oacc = None

        load_input(0)
        hpre = l1mm(0)

        for t in range(n_iters):
            h = hp.tile([128, CHUNK], F32)
            nc.scalar.activation(h[:], hpre[:], AF.Gelu, bias=b1c, scale=1.0)

            if t % GIN == 0 and t // GIN + 1 < n_iters // GIN:
                load_input(t // GIN + 1)

            if t + 1 < n_iters:
                hpre = l1mm(t + 1)

            l2o = l2p.tile([128, NT * NBITS], F32)
            for g in range(NT):
                nc.tensor.matmul(
                    l2o[:, NBITS * g:NBITS * (g + 1)],
                    h[:, 128 * g:128 * (g + 1)],
                    w2e[:],
                    start=True, stop=True, tile_position=(0, 0),
                )

            bits = bitp.tile([128, NT * NBITS], BF16)
            nc.vector.tensor_tensor(
                bits[:].rearrange("p (g b) -> p g b", b=NBITS),
                l2o[:].rearrange("p (g b) -> p g b", b=NBITS),
                theta_b,
                op=ALU.is_gt,
            )

            tmp = tmpp.tile([128, NT * NBITS], BF16)
            nc.gpsimd.tensor_tensor(
                tmp[:].rearrange("p (s b) -> p s b", b=SEG),
                bits[:].rearrange("p (s b) -> p s b", b=SEG),
                pw2_b,
                op=ALU.mult,
            )

            if t % GOUT == 0:
                oacc = oaccp.tile([128, GOUT * 2 * NT], F32)
            nc.vector.tensor_reduce(
                oacc[:, 2 * NT * (t % GOUT):2 * NT * (t % GOUT + 1)],
                tmp[:].rearrange("p (s b) -> p s b", b=SEG),
                axis=AX.X, op=ALU.add,
            )
            if t % GOUT == GOUT - 1:
                nc.sync.dma_start(outp[t // GOUT], oacc[:])

    return nc


def make_const_inputs(W1, b1, W2, b2):
    import ml_dtypes

    w1 = np.ascontiguousarray(W1[0:32, :], dtype=np.float32)
    hi = w1.astype(ml_dtypes.bfloat16)
    mid = (w1 - hi.astype(np.float32)).astype(ml_dtypes.bfloat16)
    lo = (w1 - hi.astype(np.float32) - mid.astype(np.float32)).astype(
        ml_dtypes.bfloat16
    )
    w1b = np.zeros((128, 128), dtype=ml_dtypes.bfloat16)
    w1b[0:32] = hi
    w1b[32:64] = mid
    w1b[64:96] = lo

    cst = np.zeros((128, NCST), dtype=np.float32)
    cst[:, C_W1:C_W1 + 64] = np.ascontiguousarray(w1b).view(np.float32)
    cst[:, C_W2:C_W2 + NBITS] = np.asarray(W2[:, :NBITS], dtype=np.float32)
    th = (0.5 - np.asarray(b2[:NBITS], dtype=np.float32))[None, :]
    cst[:, C_TH:C_TH + NBITS] = np.broadcast_to(th, (128, NBITS))
    pw = np.zeros((128, 14), dtype=ml_dtypes.bfloat16)
    pw[:, 0:SEG] = np.asarray([float(1 << i) for i in range(SEG)],
                              dtype=ml_dtypes.bfloat16)[None, :]
    cst[:, C_PW:C_PW + 7] = np.ascontiguousarray(pw).view(np.float32)
    cst[:, C_B1] = np.asarray(b1, dtype=np.float32)
    return {"cst": cst}


def make_bit_planes(virtual_addr, n_iters: int = N_ITERS):
    """Per-core [n_iters//GIN, 96, GIN*1024] bf16 0/1 planes.

    Partition 32s + k (s = 0..2 replication) of group tt, col m =
    bit k of addr (GIN*1024*tt + m).
    """
    import ml_dtypes

    va32 = np.asarray(virtual_addr).astype(np.uint32)
    per = n_iters * CHUNK
    ncores = va32.size // per
    out = []
    for c in range(ncores):
        seg = va32[c * per:(c + 1) * per]
        byt = seg.view(np.uint8).reshape(n_iters // GIN, GIN * CHUNK, 4)
        bits = np.unpackbits(byt, axis=-1, bitorder="little")
        pl = bits.transpose(0, 2, 1)                  # [tt, 32, GIN*CHUNK]
        pl3 = np.concatenate([pl, pl, pl], axis=1)    # [tt, 96, GIN*CHUNK]
        out.append(np.ascontiguousarray(pl3).astype(ml_dtypes.bfloat16))
    return out


def combine_output(o, n_iters: int = N_ITERS):
    """[n_iters//GOUT, 128, GOUT*16] f32 -> [per] int64.

    o[tt, a, 16*j + 2*g + half]: chunk t = GOUT*tt + j, tile g, addr
    1024*t + 128*g + a; half 0 = bits 0-12, half 1 = bits 13-25.
    """
    arr = o.reshape(n_iters // GOUT, 128, GOUT, NT, 2)
    lo = arr[..., 0].astype(np.int64)                 # [tt, a, j, g]
    hi = arr[..., 1].astype(np.int64)
    v = lo + 8192 * hi                                # [tt, a, j, g]
    v = v.transpose(0, 2, 3, 1)                       # [tt, j, g, a]
    return v.reshape(-1)


_NC_CACHE = {}
TRACE = False
LAST_RES = None


def kernel(virtual_addr, W1, b1, W2, b2):
    global LAST_RES
    if "nc" not in _NC_CACHE:
        nc = build_nc(N_ITERS)
        nc.finalize()
        _NC_CACHE["nc"] = nc
    nc = _NC_CACHE["nc"]

    consts = make_const_inputs(W1, b1, W2, b2)
    planes = make_bit_planes(virtual_addr, N_ITERS)
    in_maps = [{"bp": planes[c], **consts} for c in range(NCORES)]

    res = bass_utils.run_bass_kernel_spmd(
        nc, in_maps, list(range(NCORES)), trace=TRACE
    )
    LAST_RES = res

    outs = [combine_output(res.results[c]["outp"]) for c in range(NCORES)]
    return np.concatenate(outs)


# revision 23
# speedup vs baseline: 2.1467x; 1.3613x over previous
"""NeuralMMU Trainium2 kernel (v4 — ACT-bound, variable chunks).

Per core: 131072 addrs = 256 blocks of 512, processed in chunks of
CHUNK_BLOCKS[t] blocks ([1,1,1] + [3]*84 + [1]): small chunks at both
ends for fast pipeline fill/drain, 1536-addr chunks in steady state to
amortize the ACT per-instruction overhead (222 cyc SBUF access).

  1. Host sends bit planes as bf16 (0/1), replicated x3 for the exact
     3-way bf16 split of W1 -> SBUF [96, 8192] per 16-block DMA group.
  2. L1 (PE): one bf16 matmul k=96 per 512-block: bits @ W1 splits
     -> PSUM hpre [128, chunk] f32 (exact f32).  Emitted two chunks
     ahead so it never sits behind L2 in the in-order PE queue.
  3. ACT Gelu(+b1): PSUM -> SBUF h f32.  Bottleneck engine:
     (chunk + 222) cyc @ 1.2 GHz per chunk.
  4. L2 transposed (PE): per 128-addr tile, h-slice [128,128] f32 is
     the STATIONARY and W2[:, :26] f32 the 26-col MOVING operand (fp32
     4 cyc/row applies to only 26 cols) -> l2o [128 addr, NT*26] PSUM,
     exact f32.
  5. DVE is_gt vs theta=(0.5-b2): bits bf16; DVE mult by 2^j (13-bit
     halves); DVE segmented reduce [128, 2NT, 13] -> [128, 2NT] f32
     packed lo13/hi13 per addr tile.
  6. Per-chunk DMA -> outp[128, 2048] col slice; host: lo + 8192*hi.

PSUM: hpre 2 bufs x 3 banks + l2o 2 bufs x 1 bank = 8 banks.
"""

import numpy as np
from contextlib import ExitStack

import concourse.bass as bass
import concourse.mybir as mybir
import concourse.tile as tile
from concourse import bacc, bass_utils

B = 1_048_576
NCORES = 8
PER = B // NCORES          # 131072 addrs per core
BLK = 512
NBLKS = PER // BLK         # 256 blocks
CHUNK_BLOCKS = [1, 1, 1] + [3] * 84 + [1]
assert sum(CHUNK_BLOCKS) == NBLKS
SOFF = np.cumsum([0] + CHUNK_BLOCKS).tolist()   # block offset per chunk
N_CHUNKS = len(CHUNK_BLOCKS)
MAXNB = max(CHUNK_BLOCKS)
GINB = 16                  # blocks per input DMA group
NGRP = NBLKS // GINB       # 16 groups
GOUT = 8                   # chunks per output DMA group
NBITS = 26
SEG = 13

F32 = mybir.dt.float32
BF16 = mybir.dt.bfloat16
AF = mybir.ActivationFunctionType
ALU = mybir.AluOpType
AX = mybir.AxisListType

# cst columns (f32): w1b bf16 [128,128] = 64 | w2e f32 26 | theta f32 26
# | pw2 bf16 [128,13] -> 7 (13 bf16 + pad) | b1 1
C_W1 = 0
C_W2 = 64
C_TH = 90
C_PW = 116
C_B1 = 123
NCST = 124


def build_nc() -> bass.Bass:
    nc = bacc.Bacc("TRN2")

    bp = nc.dram_tensor("bp", [NGRP, 96, GINB * BLK], BF16,
                        kind="ExternalInput")
    cst_d = nc.dram_tensor("cst", [128, NCST], F32, kind="ExternalInput")
    outp = nc.dram_tensor("outp", [128, 8 * NBLKS], F32,
                          kind="ExternalOutput")

    with ExitStack() as ctx:
        tc = ctx.enter_context(tile.TileContext(nc))
        const = ctx.enter_context(tc.tile_pool(name="const", bufs=1))
        rpool = ctx.enter_context(tc.tile_pool(name="rp", bufs=4))
        hp = ctx.enter_context(tc.tile_pool(name="hp", bufs=2))
        bitp = ctx.enter_context(tc.tile_pool(name="bitp", bufs=2))
        tmpp = ctx.enter_context(tc.tile_pool(name="tmpp", bufs=2))
        ocp = ctx.enter_context(tc.tile_pool(name="ocp", bufs=2))
        hprep = ctx.enter_context(tc.tile_pool(name="hprep", bufs=2,
                                               space="PSUM"))
        l2p = ctx.enter_context(tc.tile_pool(name="l2p", bufs=2, space="PSUM"))

        cst = const.tile([128, NCST], F32)
        w1b = cst[:, C_W1:C_W1 + 64].bitcast(BF16)   # [128,128]; rows 0-95
        w2e = cst[:, C_W2:C_W2 + NBITS]              # [128, 26] f32
        theta = cst[:, C_TH:C_TH + NBITS]            # [128, 26] f32
        pw2 = cst[:, C_PW:C_PW + 7].bitcast(BF16)[:, 0:SEG]   # [128, 13]
        b1c = cst[:, C_B1:C_B1 + 1]

        theta_b = {nb: theta.unsqueeze(1).broadcast_to([128, 4 * nb, NBITS])
                   for nb in set(CHUNK_BLOCKS)}
        pw2_b = {nb: pw2.unsqueeze(1).broadcast_to([128, 8 * nb, SEG])
                 for nb in set(CHUNK_BLOCKS)}

        Rtiles = {}

        def load_group(g, pieces=None):
            R = rpool.tile([96, GINB * BLK], BF16)
            Rtiles[g] = R
            old = [k for k in Rtiles if k < g - 3]
            for k in old:
                del Rtiles[k]
            for b0, b1 in pieces or [(0, GINB)]:
                nc.sync.dma_start(
                    R[:, BLK * b0:BLK * b1], bp[g, :, BLK * b0:BLK * b1]
                )

        nc.sync.dma_start(cst[:], cst_d[:])
        # group 0 in pieces sized to feed the small leading chunks asap
        load_group(0, pieces=[(0, 1), (1, 4), (4, 16)])

        def l1mm(t):
            nb = CHUNK_BLOCKS[t]
            s = SOFF[t]
            glast = (s + nb - 1) // GINB
            for gpre in (glast + 1, glast + 2):
                if gpre < NGRP and gpre not in Rtiles:
                    load_group(gpre)
            hpre = hprep.tile([128, MAXNB * BLK], F32)
            for j in range(nb):
                b = s + j
                nc.tensor.matmul(
                    hpre[:, BLK * j:BLK * (j + 1)],
                    w1b[0:96, :],
                    Rtiles[b // GINB][0:96, BLK * (b % GINB):BLK * (b % GINB + 1)],
                    start=True, stop=True, tile_position=(0, 0),
                )
            return hpre

        hpres = {0: l1mm(0), 1: l1mm(1)}
        oc = None
        gofs = 0

        for t in range(N_CHUNKS):
            nb = CHUNK_BLOCKS[t]
            nt = 4 * nb                      # 128-addr tiles in this chunk
            chunk = nb * BLK

            h = hp.tile([128, MAXNB * BLK], F32)
            nc.scalar.activation(h[:, 0:chunk], hpres.pop(t)[:, 0:chunk],
                                 AF.Gelu, bias=b1c, scale=1.0)

            l2o = l2p.tile([128, 4 * MAXNB * NBITS], F32)
            for g in range(nt):
                nc.tensor.matmul(
                    l2o[:, NBITS * g:NBITS * (g + 1)],
                    h[:, 128 * g:128 * (g + 1)],
                    w2e[:],
                    start=True, stop=True, tile_position=(0, 0),
                )

            if t + 2 < N_CHUNKS:
                hpres[t + 2] = l1mm(t + 2)

            bits = bitp.tile([128, 4 * MAXNB * NBITS], BF16)
            nc.vector.tensor_tensor(
                bits[:, 0:nt * NBITS].rearrange("p (g b) -> p g b", b=NBITS),
                l2o[:, 0:nt * NBITS].rearrange("p (g b) -> p g b", b=NBITS),
                theta_b[nb],
                op=ALU.is_gt,
            )

            tmp = tmpp.tile([128, 4 * MAXNB * NBITS], BF16)
            nc.vector.tensor_tensor(
                tmp[:, 0:nt * NBITS].rearrange("p (s b) -> p s b", b=SEG),
                bits[:, 0:nt * NBITS].rearrange("p (s b) -> p s b", b=SEG),
                pw2_b[nb],
                op=ALU.mult,
            )

            if t % GOUT == 0:
                oc = ocp.tile([128, 8 * MAXNB * GOUT], F32)
                gofs = 8 * SOFF[t]
            o0 = 8 * SOFF[t] - gofs
            nc.vector.tensor_reduce(
                oc[:, o0:o0 + 2 * nt],
                tmp[:, 0:nt * NBITS].rearrange("p (s b) -> p s b", b=SEG),
                axis=AX.X, op=ALU.add,
            )
            if t % GOUT == GOUT - 1 or t == N_CHUNKS - 1:
                nc.sync.dma_start(
                    outp[:, gofs:gofs + o0 + 2 * nt], oc[:, 0:o0 + 2 * nt]
                )

    return nc


def make_const_inputs(W1, b1, W2, b2):
    import ml_dtypes

    w1 = np.ascontiguousarray(W1[0:32, :], dtype=np.float32)
    hi = w1.astype(ml_dtypes.bfloat16)
    mid = (w1 - hi.astype(np.float32)).astype(ml_dtypes.bfloat16)
    lo = (w1 - hi.astype(np.float32) - mid.astype(np.float32)).astype(
        ml_dtypes.bfloat16
    )
    w1b = np.zeros((128, 128), dtype=ml_dtypes.bfloat16)
    w1b[0:32] = hi
    w1b[32:64] = mid
    w1b[64:96] = lo

    cst = np.zeros((128, NCST), dtype=np.float32)
    cst[:, C_W1:C_W1 + 64] = np.ascontiguousarray(w1b).view(np.float32)
    cst[:, C_W2:C_W2 + NBITS] = np.asarray(W2[:, :NBITS], dtype=np.float32)
    th = (0.5 - np.asarray(b2[:NBITS], dtype=np.float32))[None, :]
    cst[:, C_TH:C_TH + NBITS] = np.broadcast_to(th, (128, NBITS))
    pw = np.zeros((128, 14), dtype=ml_dtypes.bfloat16)
    pw[:, 0:SEG] = np.asarray([float(1 << i) for i in range(SEG)],
                              dtype=ml_dtypes.bfloat16)[None, :]
    cst[:, C_PW:C_PW + 7] = np.ascontiguousarray(pw).view(np.float32)
    cst[:, C_B1] = np.asarray(b1, dtype=np.float32)
    return {"cst": cst}


def make_bit_planes(virtual_addr):
    """Per-core [NGRP, 96, 8192] bf16 0/1 planes.

    Partition 32s + k (s = 0..2 replication) of group g, col m =
    bit k of addr (8192*g + m) within the core's address range.
    """
    import ml_dtypes

    va32 = np.asarray(virtual_addr).astype(np.uint32)
    ncores = va32.size // PER
    out = []
    for c in range(ncores):
        seg = va32[c * PER:(c + 1) * PER]
        byt = seg.view(np.uint8).reshape(NGRP, GINB * BLK, 4)
        bits = np.unpackbits(byt, axis=-1, bitorder="little")
        pl = bits.transpose(0, 2, 1)                  # [g, 32, 8192]
        pl3 = np.concatenate([pl, pl, pl], axis=1)    # [g, 96, 8192]
        out.append(np.ascontiguousarray(pl3).astype(ml_dtypes.bfloat16))
    return out


def combine_output(o):
    """[128, 2048] f32 -> [PER] int64.

    Column 2k / 2k+1 = lo13 / hi13 of global 128-addr tile k;
    addr = 128*k + partition.
    """
    lo = o[:, 0::2].astype(np.int64)      # [128, 1024]
    hi = o[:, 1::2].astype(np.int64)
    v = lo + 8192 * hi
    return v.T.reshape(-1)


_NC_CACHE = {}
TRACE = False
LAST_RES = None


def kernel(virtual_addr, W1, b1, W2, b2):
    global LAST_RES
    if "nc" not in _NC_CACHE:
        nc = build_nc()
        nc.finalize()
        _NC_CACHE["nc"] = nc
    nc = _NC_CACHE["nc"]

    consts = make_const_inputs(W1, b1, W2, b2)
    planes = make_bit_planes(virtual_addr)
    in_maps = [{"bp": planes[c], **consts} for c in range(NCORES)]

    res = bass_utils.run_bass_kernel_spmd(
        nc, in_maps, list(range(NCORES)), trace=TRACE
    )
    LAST_RES = res

    outs = [combine_output(res.results[c]["outp"]) for c in range(NCORES)]
    return np.concatenate(outs)


# revision 28
# speedup vs baseline: 2.1548x; 1.0038x over previous
"""NeuralMMU Trainium2 kernel (v4 — ACT-bound, variable chunks).

Per core: 131072 addrs = 256 blocks of 512, processed in chunks of
CHUNK_BLOCKS[t] blocks ([1,1,1] + [3]*84 + [1]): small chunks at both
ends for fast pipeline fill/drain, 1536-addr chunks in steady state to
amortize the ACT per-instruction overhead (222 cyc SBUF access).

  1. Host sends bit planes as bf16 (0/1), replicated x3 for the exact
     3-way bf16 split of W1 -> SBUF [96, 8192] per 16-block DMA group.
  2. L1 (PE): one bf16 matmul k=96 per 512-block: bits @ W1 splits
     -> PSUM hpre [128, chunk] f32 (exact f32).  Emitted two chunks
     ahead so it never sits behind L2 in the in-order PE queue.
  3. ACT Gelu(+b1): PSUM -> SBUF h f32.  Bottleneck engine:
     (chunk + 222) cyc @ 1.2 GHz per chunk.
  4. L2 transposed (PE): per 128-addr tile, h-slice [128,128] f32 is
     the STATIONARY and W2[:, :26] f32 the 26-col MOVING operand (fp32
     4 cyc/row applies to only 26 cols) -> l2o [128 addr, NT*26] PSUM,
     exact f32.
  5. DVE is_gt vs theta=(0.5-b2): bits bf16; DVE mult by 2^j (13-bit
     halves); DVE segmented reduce [128, 2NT, 13] -> [128, 2NT] f32
     packed lo13/hi13 per addr tile.
  6. Per-chunk DMA -> outp[128, 2048] col slice; host: lo + 8192*hi.

PSUM: hpre 2 bufs x 3 banks + l2o 2 bufs x 1 bank = 8 banks.
"""

import numpy as np
from contextlib import ExitStack

import concourse.bass as bass
import concourse.mybir as mybir
import concourse.tile as tile
from concourse import bacc, bass_utils

B = 1_048_576
NCORES = 8
PER = B // NCORES          # 131072 addrs per core
BLK = 512
NBLKS = PER // BLK         # 256 blocks
CHUNK_BLOCKS = [2, 2] + [3] * 84
assert sum(CHUNK_BLOCKS) == NBLKS
SOFF = np.cumsum([0] + CHUNK_BLOCKS).tolist()   # block offset per chunk
N_CHUNKS = len(CHUNK_BLOCKS)
MAXNB = max(CHUNK_BLOCKS)
GINB = 16                  # blocks per input DMA group
NGRP = NBLKS // GINB       # 16 groups
GOUT = 8                   # chunks per output DMA group
NBITS = 26
SEG = 13

F32 = mybir.dt.float32
BF16 = mybir.dt.bfloat16
AF = mybir.ActivationFunctionType
ALU = mybir.AluOpType
AX = mybir.AxisListType

# cst columns (f32): w1b bf16 [128,128] = 64 | w2e f32 26 | theta f32 26
# | pw2 bf16 [128,13] -> 7 (13 bf16 + pad) | b1 1
C_W1 = 0
C_W2 = 64
C_TH = 90
C_PW = 116
C_B1 = 123
NCST = 124


def build_nc() -> bass.Bass:
    nc = bacc.Bacc("TRN2")

    bp = nc.dram_tensor("bp", [NGRP, 96, GINB * BLK], BF16,
                        kind="ExternalInput")
    cst_d = nc.dram_tensor("cst", [128, NCST], F32, kind="ExternalInput")
    outp = nc.dram_tensor("outp", [128, 8 * NBLKS], F32,
                          kind="ExternalOutput")

    with ExitStack() as ctx:
        tc = ctx.enter_context(tile.TileContext(nc))
        const = ctx.enter_context(tc.tile_pool(name="const", bufs=1))
        rpool = ctx.enter_context(tc.tile_pool(name="rp", bufs=4))
        hp = ctx.enter_context(tc.tile_pool(name="hp", bufs=3))
        bitp = ctx.enter_context(tc.tile_pool(name="bitp", bufs=2))
        tmpp = ctx.enter_context(tc.tile_pool(name="tmpp", bufs=2))
        ocp = ctx.enter_context(tc.tile_pool(name="ocp", bufs=2))
        hprep = ctx.enter_context(tc.tile_pool(name="hprep", bufs=2,
                                               space="PSUM"))
        l2p = ctx.enter_context(tc.tile_pool(name="l2p", bufs=2, space="PSUM"))

        cst = const.tile([128, NCST], F32)
        w1b = cst[:, C_W1:C_W1 + 64].bitcast(BF16)   # [128,128]; rows 0-95
        w2e = cst[:, C_W2:C_W2 + NBITS]              # [128, 26] f32
        theta = cst[:, C_TH:C_TH + NBITS]            # [128, 26] f32
        pw2 = cst[:, C_PW:C_PW + 7].bitcast(BF16)[:, 0:SEG]   # [128, 13]
        b1c = cst[:, C_B1:C_B1 + 1]

        theta_b = {nb: theta.unsqueeze(1).broadcast_to([128, 4 * nb, NBITS])
                   for nb in set(CHUNK_BLOCKS)}
        pw2_b = {nb: pw2.unsqueeze(1).broadcast_to([128, 8 * nb, SEG])
                 for nb in set(CHUNK_BLOCKS)}

        Rtiles = {}

        def load_group(g, pieces=None):
            R = rpool.tile([96, GINB * BLK], BF16)
            Rtiles[g] = R
            old = [k for k in Rtiles if k < g - 3]
            for k in old:
                del Rtiles[k]
            for b0, b1 in pieces or [(0, GINB)]:
                nc.sync.dma_start(
                    R[:, BLK * b0:BLK * b1], bp[g, :, BLK * b0:BLK * b1]
                )

        nc.sync.dma_start(cst[:], cst_d[:])
        # group 0 in pieces sized to feed the small leading chunks asap
        load_group(0, pieces=[(0, 2), (2, 4), (4, 7), (7, 10), (10, 16)])

        def l1mm(t):
            nb = CHUNK_BLOCKS[t]
            s = SOFF[t]
            glast = (s + nb - 1) // GINB
            for gpre in (glast + 1, glast + 2):
                if gpre < NGRP and gpre not in Rtiles:
                    load_group(gpre)
            hpre = hprep.tile([128, MAXNB * BLK], F32)
            for j in range(nb):
                b = s + j
                nc.tensor.matmul(
                    hpre[:, BLK * j:BLK * (j + 1)],
                    w1b[0:96, :],
                    Rtiles[b // GINB][0:96, BLK * (b % GINB):BLK * (b % GINB + 1)],
                    start=True, stop=True, tile_position=(0, 0),
                )
            return hpre

        hpres = {0: l1mm(0), 1: l1mm(1)}
        oc = None
        gofs = 0
        flushed = 0

        for t in range(N_CHUNKS):
            nb = CHUNK_BLOCKS[t]
            nt = 4 * nb                      # 128-addr tiles in this chunk
            chunk = nb * BLK

            h = hp.tile([128, MAXNB * BLK], F32)
            nc.scalar.activation(h[:, 0:chunk], hpres.pop(t)[:, 0:chunk],
                                 AF.Gelu, bias=b1c, scale=1.0)

            l2o = l2p.tile([128, 4 * MAXNB * NBITS], F32)
            for g in range(nt):
                nc.tensor.matmul(
                    l2o[:, NBITS * g:NBITS * (g + 1)],
                    h[:, 128 * g:128 * (g + 1)],
                    w2e[:],
                    start=True, stop=True, tile_position=(0, 0),
                )

            if t + 2 < N_CHUNKS:
                hpres[t + 2] = l1mm(t + 2)

            bits = bitp.tile([128, 4 * MAXNB * NBITS], BF16)
            nc.vector.tensor_tensor(
                bits[:, 0:nt * NBITS].rearrange("p (g b) -> p g b", b=NBITS),
                l2o[:, 0:nt * NBITS].rearrange("p (g b) -> p g b", b=NBITS),
                theta_b[nb],
                op=ALU.is_gt,
            )

            tmp = tmpp.tile([128, 4 * MAXNB * NBITS], BF16)
            nc.vector.tensor_tensor(
                tmp[:, 0:nt * NBITS].rearrange("p (s b) -> p s b", b=SEG),
                bits[:, 0:nt * NBITS].rearrange("p (s b) -> p s b", b=SEG),
                pw2_b[nb],
                op=ALU.mult,
            )

            if t % GOUT == 0:
                oc = ocp.tile([128, 8 * MAXNB * GOUT], F32)
                gofs = 8 * SOFF[t]
                flushed = 0
            o0 = 8 * SOFF[t] - gofs
            nc.vector.tensor_reduce(
                oc[:, o0:o0 + 2 * nt],
                tmp[:, 0:nt * NBITS].rearrange("p (s b) -> p s b", b=SEG),
                axis=AX.X, op=ALU.add,
            )
            if t % GOUT == GOUT - 1 or t >= N_CHUNKS - 2:
                nc.sync.dma_start(
                    outp[:, gofs + flushed:gofs + o0 + 2 * nt],
                    oc[:, flushed:o0 + 2 * nt],
                )
                flushed = o0 + 2 * nt

    return nc


def make_const_inputs(W1, b1, W2, b2):
    import ml_dtypes

    w1 = np.ascontiguousarray(W1[0:32, :], dtype=np.float32)
    hi = w1.astype(ml_dtypes.bfloat16)
    mid = (w1 - hi.astype(np.float32)).astype(ml_dtypes.bfloat16)
    lo = (w1 - hi.astype(np.float32) - mid.astype(np.float32)).astype(
        ml_dtypes.bfloat16
    )
    w1b = np.zeros((128, 128), dtype=ml_dtypes.bfloat16)
    w1b[0:32] = hi
    w1b[32:64] = mid
    w1b[64:96] = lo

    cst = np.zeros((128, NCST), dtype=np.float32)
    cst[:, C_W1:C_W1 + 64] = np.ascontiguousarray(w1b).view(np.float32)
    cst[:, C_W2:C_W2 + NBITS] = np.asarray(W2[:, :NBITS], dtype=np.float32)
    th = (0.5 - np.asarray(b2[:NBITS], dtype=np.float32))[None, :]
    cst[:, C_TH:C_TH + NBITS] = np.broadcast_to(th, (128, NBITS))
    pw = np.zeros((128, 14), dtype=ml_dtypes.bfloat16)
    pw[:, 0:SEG] = np.asarray([float(1 << i) for i in range(SEG)],
                              dtype=ml_dtypes.bfloat16)[None, :]
    cst[:, C_PW:C_PW + 7] = np.ascontiguousarray(pw).view(np.float32)
    cst[:, C_B1] = np.asarray(b1, dtype=np.float32)
    return {"cst": cst}


def make_bit_planes(virtual_addr):
    """Per-core [NGRP, 96, 8192] bf16 0/1 planes.

    Partition 32s + k (s = 0..2 replication) of group g, col m =
    bit k of addr (8192*g + m) within the core's address range.
    """
    import ml_dtypes

    va32 = np.asarray(virtual_addr).astype(np.uint32)
    ncores = va32.size // PER
    out = []
    for c in range(ncores):
        seg = va32[c * PER:(c + 1) * PER]
        byt = seg.view(np.uint8).reshape(NGRP, GINB * BLK, 4)
        bits = np.unpackbits(byt, axis=-1, bitorder="little")
        pl = bits.transpose(0, 2, 1)                  # [g, 32, 8192]
        pl3 = np.concatenate([pl, pl, pl], axis=1)    # [g, 96, 8192]
        out.append(np.ascontiguousarray(pl3).astype(ml_dtypes.bfloat16))
    return out


def combine_output(o):
    """[128, 2048] f32 -> [PER] int64.

    Column 2k / 2k+1 = lo13 / hi13 of global 128-addr tile k;
    addr = 128*k + partition.
    """
    lo = o[:, 0::2].astype(np.int64)      # [128, 1024]
    hi = o[:, 1::2].astype(np.int64)
    v = lo + 8192 * hi
    return v.T.reshape(-1)


_NC_CACHE = {}
TRACE = False
LAST_RES = None


def kernel(virtual_addr, W1, b1, W2, b2):
    global LAST_RES
    if "nc" not in _NC_CACHE:
        nc = build_nc()
        nc.finalize()
        _NC_CACHE["nc"] = nc
    nc = _NC_CACHE["nc"]

    consts = make_const_inputs(W1, b1, W2, b2)
    planes = make_bit_planes(virtual_addr)
    in_maps = [{"bp": planes[c], **consts} for c in range(NCORES)]

    res = bass_utils.run_bass_kernel_spmd(
        nc, in_maps, list(range(NCORES)), trace=TRACE
    )
    LAST_RES = res

    outs = [combine_output(res.results[c]["outp"]) for c in range(NCORES)]
    return np.concatenate(outs)
